# revision 33
# baseline (speedup 1.0000x reference)
"""BloomWISARD forward on 8 trn2 NeuronCores — full device pipeline.

Sharding: data-parallel over batch (samples sharded 8 ways); bloom filters
bit-packed on host (BLAS gemm trick), shipped sharded by (class, neuron) and
AllGathered on-device so each core holds all 10x128 packed filter tables
(80KB/partition, SBUF-resident).

Per (class, batch-chunk-of-128) on each core:
  1. ap_gather (GPSIMD): permutation — gather the 32-bit sample words
     containing each of the 4096 permuted bit positions (idx = tm>>5,
     shared across partitions since partition dim = batch).
  2. DVE: extract bits ((w >> (tm&31)) & 1), then H3 hash per hash fn k:
     mult by hash_matrix pattern + segmented XOR-reduce over tuple dim.
  3. PE: transpose h (batch-major -> neuron-major) via identity matmul.
  4. ap_gather (GPSIMD): bloom probe — gather packed filter words
     (partition dim = neuron, per-core streams interleave the 16 neurons
     of each Q7 core), diagonal-extract, test bit, AND over 4 hashes.
  5. TensorE: count over neurons = ones-vector matmul (partition reduce).
"""
import numpy as np

B = 8192
ENTRY = 4096
C = 10
T = 32
N = ENTRY // T  # 128
F = 65536
H = 4
NCORES = 8
BC = B // NCORES       # 1024 samples per core
NCHUNK = BC // 128     # 8 chunks of 128 samples
SWORDS = ENTRY // 32   # 128 sample words per sample
FWORDS = F // 32       # 2048 filter words per (class, neuron)
FSH = C * N // NCORES  # 160 packed-filter rows per core shard
SM_LEN = 59520         # packed small-constants buffer, in int32 words

_CACHE = {}


def _build_nc():
    import concourse.bacc as bacc
    import concourse.mybir as mybir
    import concourse.tile as tile
    from contextlib import ExitStack

    dt = mybir.dt
    nc = bacc.Bacc("TRN2", target_bir_lowering=False, debug=False,
                   num_devices=NCORES)

    sw_d = nc.dram_tensor("sw", [BC, SWORDS], dt.int32, kind="ExternalInput")
    fw_d = nc.dram_tensor("fw", [FSH, FWORDS], dt.int32, kind="ExternalInput")
    sm_d = nc.dram_tensor("sm", [1, SM_LEN], dt.int32, kind="ExternalInput")
    resp_d = nc.dram_tensor("resp", [1, C * BC], dt.float32,
                            kind="ExternalOutput")
    # offsets (in i32 words) into the packed smalls buffer
    ixp_ap = sm_d.ap()[0:1, 0:20480].bitcast(dt.int16) \
        .rearrange("o (r s) -> (o r) s", s=256)            # [160, 256] i16
    hmr_ap = sm_d.ap()[0:1, 20480:36864] \
        .rearrange("o (p q) -> (o p) q", q=H * T)          # [128, 128] i32
    shw_ap = sm_d.ap()[0:1, 36864:40960] \
        .rearrange("o (p q) -> (o p) q", q=T)              # [128, 32] i32
    msk_ap = sm_d.ap()[0:1, 40960:43008] \
        .rearrange("o (p q) -> (o p) q", q=16)             # [128, 16] i32
    idn_ap = sm_d.ap()[0:1, 43008:59392].bitcast(dt.float32) \
        .rearrange("o (p q) -> (o p) q", q=128)            # [128, 128] f32
    one_ap = sm_d.ap()[0:1, 59392:59520].bitcast(dt.float32) \
        .rearrange("o (p q) -> (o p) q", q=1)              # [128, 1] f32

    with tile.TileContext(nc) as tc:
        with ExitStack() as ctx:
            dram = ctx.enter_context(tc.tile_pool(name="dram", bufs=1,
                                                  space="DRAM"))
            pool = ctx.enter_context(tc.tile_pool(name="main", bufs=1))
            work = ctx.enter_context(tc.tile_pool(name="work", bufs=2))
            ppool = ctx.enter_context(tc.tile_pool(name="ps", bufs=2,
                                                   space="PSUM"))

            # --- filters: shard -> AllGather -> SBUF-resident packed tables
            # The emulated runtime only delivers the first half of each
            # cross-pair contribution, so send the shard twice and read the
            # (complete) first half of each core's doubled contribution.
            fb2 = dram.tile([2 * FSH, FWORDS], dt.int32)
            ffx = nc.dram_tensor("ffx", [2 * C * N, FWORDS], dt.int32,
                                 addr_space="Shared")
            nc.gpsimd.dma_start(fb2[0:FSH, :], fw_d.ap())
            nc.gpsimd.dma_start(fb2[FSH:2 * FSH, :], fw_d.ap())
            nc.gpsimd.collective_compute(
                "AllGather", mybir.AluOpType.bypass,
                replica_groups=[list(range(NCORES))],
                ins=[fb2[:].opt()], outs=[ffx.ap().opt()])

            filt = []
            for c in range(C):
                ft = pool.tile([128, FWORDS], dt.int32, name=f"filt{c}")
                # global packed row r -> core k=r//FSH at ffx row 2*FSH*k + r%FSH
                r = c * N
                while r < (c + 1) * N:
                    k = r // FSH
                    end = min((c + 1) * N, (k + 1) * FSH)
                    nc.sync.dma_start(
                        ft[r - c * N:end - c * N, :],
                        ffx.ap()[2 * FSH * k + (r % FSH):
                                 2 * FSH * k + (r % FSH) + (end - r), :])
                    r = end
                filt.append(ft)

            # --- small constants (one packed buffer)
            ixp = []
            for c in range(C):
                it_ = pool.tile([128, 256], dt.int16, name=f"ixp{c}")
                for g in range(8):
                    nc.sync.dma_start(it_[16 * g:16 * (g + 1), :],
                                      ixp_ap[c * 16:(c + 1) * 16, :])
                ixp.append(it_)
            hmr = pool.tile([128, H * T], dt.int32, name="hmr")
            nc.sync.dma_start(hmr[:], hmr_ap)
            shw = pool.tile([128, T], dt.int32, name="shw")
            nc.sync.dma_start(shw[:], shw_ap)
            msk = pool.tile([128, 16], dt.int32, name="msk")
            nc.sync.dma_start(msk[:], msk_ap)
            idn = pool.tile([128, 128], dt.float32, name="idn")
            nc.sync.dma_start(idn[:], idn_ap)
            one = pool.tile([128, 1], dt.float32, name="one")
            nc.sync.dma_start(one[:], one_ap)

            for t in range(NCHUNK):
                # unpack this chunk's sample words into bit-granular table
                wtab = work.tile([128, SWORDS], dt.int32, name="wtab", bufs=1)
                nc.sync.dma_start(wtab[:], sw_d.ap()[t * 128:(t + 1) * 128, :])
                btab = work.tile([128, ENTRY], dt.int32, name="btab", bufs=1)
                b3t = btab[:].rearrange("p (w j) -> p w j", j=T)
                wv = wtab[:].unsqueeze(2).broadcast_to([128, SWORDS, T])
                sv = shw[:].unsqueeze(1).broadcast_to([128, SWORDS, T])
                nc.vector.tensor_tensor(
                    b3t, wv, sv, mybir.AluOpType.logical_shift_right)
                nc.vector.tensor_scalar(
                    btab[:], btab[:], 1, None, mybir.AluOpType.bitwise_and)
                for c in range(C):
                    # 1) permutation gather: bit-granular, idx = tm directly
                    bits = work.tile([128, ENTRY], dt.int32, name="bits",
                                     bufs=1)
                    nc.gpsimd.ap_gather(
                        bits[:], btab[:], ixp[c][:],
                        channels=128, num_elems=ENTRY, d=1, num_idxs=ENTRY)
                    # 2) H3 hash
                    idxt = work.tile([128, 512], dt.int16, name="idxt", bufs=1)
                    shnt = work.tile([128, 512], dt.int32, name="shnt", bufs=1)
                    tk = work.tile([128, ENTRY], dt.int32, name="tk", bufs=1)
                    b3 = bits[:].rearrange("p (n j) -> p n j", j=T)
                    t3 = tk[:].rearrange("p (n j) -> p n j", j=T)
                    for k in range(H):
                        hk = work.tile([128, 128], dt.int32, name="hk", bufs=1)
                        hmk = hmr[:, k * T:(k + 1) * T].unsqueeze(1) \
                            .broadcast_to([128, N, T])
                        nc.vector.tensor_tensor(t3, b3, hmk,
                                                mybir.AluOpType.mult)
                        nc.vector.tensor_reduce(
                            hk[:], t3, mybir.AxisListType.X,
                            mybir.AluOpType.bitwise_xor)
                        # 3) split to word idx / bit shift, f32, PE-transpose
                        wf = work.tile([128, 128], dt.int32, name="wf", bufs=1)
                        nc.vector.tensor_scalar(
                            wf[:], hk[:], 5, None,
                            mybir.AluOpType.logical_shift_right)
                        wff = work.tile([128, 128], dt.float32, name="wff",
                                        bufs=1)
                        nc.scalar.copy(wff[:], wf[:])
                        sf = work.tile([128, 128], dt.int32, name="sf", bufs=1)
                        nc.vector.tensor_scalar(
                            sf[:], hk[:], 31, None, mybir.AluOpType.bitwise_and)
                        sff = work.tile([128, 128], dt.float32, name="sff",
                                        bufs=1)
                        nc.scalar.copy(sff[:], sf[:])
                        tw = ppool.tile([128, 128], dt.float32, name="tw")
                        nc.tensor.matmul(tw[:], wff[:], idn[:],
                                         is_transpose=True)
                        ts = ppool.tile([128, 128], dt.float32, name="ts")
                        nc.tensor.matmul(ts[:], sff[:], idn[:],
                                         is_transpose=True)
                        iv = idxt[:].rearrange("p (b k) -> p b k", k=H)
                        nc.vector.tensor_copy(iv[:, :, k], tw[:])
                        sv = shnt[:].rearrange("p (b k) -> p b k", k=H)
                        nc.scalar.copy(sv[:, :, k], ts[:])
                    # 4) bloom probes, two halves of 64 samples
                    for hf in range(2):
                        gout = work.tile([128, 4096], dt.int32, name="gout")
                        nc.gpsimd.ap_gather(
                            gout[:], filt[c][:],
                            idxt[:, hf * 256:(hf + 1) * 256],
                            channels=128, num_elems=FWORDS, d=1, num_idxs=4096)
                        gv = gout[:].rearrange("p (s j) -> p s j", j=16)
                        mb = msk[:].unsqueeze(1).broadcast_to([128, 256, 16])
                        wsel = work.tile([128, 256], dt.int32, name="wsel",
                                         bufs=1)
                        nc.vector.tensor_tensor(
                            gv, gv, mb, mybir.AluOpType.bitwise_and)
                        nc.vector.tensor_reduce(
                            wsel[:], gv, mybir.AxisListType.X,
                            mybir.AluOpType.bitwise_or)
                        nc.vector.tensor_tensor(
                            wsel[:], wsel[:],
                            shnt[:, hf * 256:(hf + 1) * 256],
                            mybir.AluOpType.logical_shift_right)
                        nc.vector.tensor_scalar(
                            wsel[:], wsel[:], 1, None,
                            mybir.AluOpType.bitwise_and)
                        # AND over the 4 hash fns
                        mem = work.tile([128, 64], dt.int32, name="mem",
                                        bufs=1)
                        nc.vector.tensor_reduce(
                            mem[:], wsel[:].rearrange("p (b k) -> p b k", k=H),
                            mybir.AxisListType.X, mybir.AluOpType.bitwise_and)
                        memf = work.tile([128, 64], dt.float32, name="memf",
                                         bufs=1)
                        nc.vector.tensor_copy(memf[:], mem[:])
                        # 5) count over neurons (partition reduce via matmul)
                        pr = ppool.tile([1, 64], dt.float32, name="pr")
                        nc.tensor.matmul(pr[:], one[:], memf[:],
                                         start=True, stop=True)
                        sr = work.tile([1, 64], dt.float32, name="sr")
                        nc.scalar.copy(sr[:], pr[:])
                        off = c * BC + t * 128 + hf * 64
                        nc.sync.dma_start(resp_d.ap()[0:1, off:off + 64],
                                          sr[:])
    nc.compile()
    return nc


def _get_runner():
    if "runner" in _CACHE:
        return _CACHE["runner"]
    import jax
    import numpy as _np
    from jax.sharding import Mesh, PartitionSpec
    from jax.experimental.shard_map import shard_map
    from concourse import bass2jax

    nc = _build_nc()
    bass2jax.install_neuronx_cc_hook()
    devices = jax.devices()[:NCORES]
    mesh = Mesh(_np.asarray(devices), ("core",))
    out_avals = [jax.core.ShapedArray((1, C * BC), _np.float32)]
    partition_name = (nc.partition_id_tensor.name
                      if nc.partition_id_tensor else None)
    in_names = ["sw", "fw", "sm", "resp"]
    if partition_name:
        in_names.append(partition_name)

    def _body(sw, fw, sm, zout):
        operands = [sw, fw, sm, zout]
        if partition_name:
            operands.append(bass2jax.partition_id_tensor())
        outs = bass2jax._bass_exec_p.bind(
            *operands,
            out_avals=tuple(out_avals),
            in_names=tuple(in_names),
            out_names=("resp",),
            lowering_input_output_aliases=(),
            sim_require_finite=True,
            sim_require_nnan=True,
            nc=nc)
        return tuple(outs)

    P = PartitionSpec
    in_specs = (P("core"), P("core"), P(), P("core"))
    runner = jax.jit(
        shard_map(_body, mesh=mesh, in_specs=in_specs,
                  out_specs=(P("core"),), check_rep=False),
        donate_argnums=(3,), keep_unused=True)
    _CACHE["runner"] = (runner, mesh)
    return _CACHE["runner"]


_PK_W = None


def _pack_bits_f32(arr_f32_flat32):
    """[M, 32] f32 of 0/1 -> [M] uint32 via BLAS gemm (exact in two 16-bit
    halves)."""
    global _PK_W
    if _PK_W is None:
        W = np.zeros((32, 2), np.float32)
        W[:16, 0] = (1 << np.arange(16)).astype(np.float32)
        W[16:, 1] = (1 << np.arange(16)).astype(np.float32)
        _PK_W = W
    lohi = arr_f32_flat32 @ _PK_W
    return lohi[:, 0].astype(np.uint32) | (lohi[:, 1].astype(np.uint32) << 16)


def _get_packers():
    """Numba single-pass bit packers; fall back to the gemm path."""
    if "packers" in _CACHE:
        return _CACHE["packers"]
    try:
        from numba import njit

        @njit(cache=False, nogil=True)
        def _pk_i32(src, out):
            # src int32 [M, 32] of 0/1 -> out uint32 [M]
            for i in range(src.shape[0]):
                w = np.uint32(0)
                for t in range(32):
                    w |= np.uint32(src[i, t]) << np.uint32(t)
                out[i] = w

        @njit(cache=False, nogil=True)
        def _pk_f32v(src, out):
            # src uint32-view of f32 0.0/1.0 [M, 32]; 1.0f has bit 29 set
            for i in range(src.shape[0]):
                w = np.uint32(0)
                for t in range(32):
                    w |= ((src[i, t] >> np.uint32(29)) & np.uint32(1)) \
                        << np.uint32(t)
                out[i] = w

        # trigger compilation now (first, untimed call)
        _pk_i32(np.zeros((2, 32), np.int32), np.empty(2, np.uint32))
        _pk_f32v(np.zeros((2, 32), np.uint32), np.empty(2, np.uint32))

        def pack_samples(samples):
            out = np.empty(B * SWORDS, np.uint32)
            _pk_i32(samples.reshape(-1, 32), out)
            return out.reshape(B, SWORDS).view(np.int32)

        def pack_filters(filters):
            out = np.empty(C * N * FWORDS, np.uint32)
            _pk_f32v(np.ascontiguousarray(filters).reshape(-1, 32)
                     .view(np.uint32), out)
            return out.reshape(C * N, FWORDS).view(np.int32)
    except Exception:
        def pack_samples(samples):
            sf = samples.astype(np.float32)
            return _pack_bits_f32(sf.reshape(-1, 32)).reshape(
                B, SWORDS).view(np.int32)

        def pack_filters(filters):
            fwv = _pack_bits_f32(np.ascontiguousarray(
                filters.reshape(-1, 32).astype(np.float32, copy=False)))
            return fwv.reshape(C * N, FWORDS).view(np.int32)

    _CACHE["packers"] = (pack_samples, pack_filters)
    return _CACHE["packers"]


def kernel(samples, tuple_mapping, hash_matrix, filters):
    samples = np.asarray(samples)
    tm = np.asarray(tuple_mapping).astype(np.int64)
    hm = np.asarray(hash_matrix).astype(np.int64)
    filters = np.asarray(filters)

    runner, mesh = _get_runner()
    pack_samples, pack_filters = _get_packers()

    # --- pack filters (f32 0/1 -> u32 words)
    fw = pack_filters(filters)

    # --- small constants, packed into one int32 buffer
    ixp = np.empty((C * 16, 256), np.int16)
    for c in range(C):
        # wrapped idx layout: stream pos i -> (partition i%16, slot i//16)
        ixp[c * 16:(c + 1) * 16, :] = tm[c].astype(np.int16).reshape(256, 16).T
    hmr = np.tile(hm.reshape(1, H * T).astype(np.int32), (128, 1))
    shw = np.tile(np.arange(T, dtype=np.int32)[None, :], (128, 1))
    pp = np.arange(128)
    msk = -(np.arange(16)[None, :] == (pp[:, None] & 15)).astype(np.int32)
    idn = np.eye(128, dtype=np.float32)
    one = np.ones((128, 1), np.float32)
    sm = np.concatenate([
        ixp.reshape(-1).view(np.int32),
        hmr.reshape(-1), shw.reshape(-1), msk.reshape(-1),
        idn.reshape(-1).view(np.int32), one.reshape(-1).view(np.int32),
    ]).reshape(1, SM_LEN)

    sw = pack_samples(samples)

    zout = np.zeros((NCORES, C * BC), np.float32)
    outs = runner(sw, fw, sm, zout)
    resp = np.asarray(outs[0])                             # [8, C*BC]

    out = np.empty((B, C), np.float32)
    r = resp.reshape(NCORES, C, BC)
    for core in range(NCORES):
        out[core * BC:(core + 1) * BC, :] = r[core].T
    return out


# revision 35
# speedup vs baseline: 1.0572x; 1.0572x over previous
"""BloomWISARD forward on 8 trn2 NeuronCores — full device pipeline.

Sharding: data-parallel over batch (samples sharded 8 ways); bloom filters
bit-packed on host (BLAS gemm trick), shipped sharded by (class, neuron) and
AllGathered on-device so each core holds all 10x128 packed filter tables
(80KB/partition, SBUF-resident).

Per (class, batch-chunk-of-128) on each core:
  1. ap_gather (GPSIMD): permutation — gather the 32-bit sample words
     containing each of the 4096 permuted bit positions (idx = tm>>5,
     shared across partitions since partition dim = batch).
  2. DVE: extract bits ((w >> (tm&31)) & 1), then H3 hash per hash fn k:
     mult by hash_matrix pattern + segmented XOR-reduce over tuple dim.
  3. PE: transpose h (batch-major -> neuron-major) via identity matmul.
  4. ap_gather (GPSIMD): bloom probe — gather packed filter words
     (partition dim = neuron, per-core streams interleave the 16 neurons
     of each Q7 core), diagonal-extract, test bit, AND over 4 hashes.
  5. TensorE: count over neurons = ones-vector matmul (partition reduce).
"""
import numpy as np

B = 8192
ENTRY = 4096
C = 10
T = 32
N = ENTRY // T  # 128
F = 65536
H = 4
NCORES = 8
BC = B // NCORES       # 1024 samples per core
NCHUNK = BC // 128     # 8 chunks of 128 samples
SWORDS = ENTRY // 32   # 128 sample words per sample
FWORDS = F // 32       # 2048 filter words per (class, neuron)
FSH = C * N // NCORES  # 160 packed-filter rows per core shard
SM_LEN = 59520         # packed small-constants buffer, in int32 words
BLOB_LEN = BC * SWORDS + FSH * FWORDS + SM_LEN  # single merged input

_CACHE = {}


def _build_nc():
    import concourse.bacc as bacc
    import concourse.mybir as mybir
    import concourse.tile as tile
    from contextlib import ExitStack

    dt = mybir.dt
    nc = bacc.Bacc("TRN2", target_bir_lowering=False, debug=False,
                   num_devices=NCORES)

    blob_d = nc.dram_tensor("blob", [1, BLOB_LEN], dt.int32,
                            kind="ExternalInput")
    resp_d = nc.dram_tensor("resp", [1, C * BC], dt.float32,
                            kind="ExternalOutput")
    SWO = 0                      # sw:  BC*SWORDS words
    FWO = BC * SWORDS            # fw:  FSH*FWORDS words
    SMO = FWO + FSH * FWORDS     # smalls: SM_LEN words

    def _sw_ap(t):
        return blob_d.ap()[0:1, SWO + t * 128 * SWORDS:
                           SWO + (t + 1) * 128 * SWORDS] \
            .rearrange("o (p w) -> (o p) w", w=SWORDS)     # [128, 128] i32

    fw_ap = blob_d.ap()[0:1, FWO:FWO + FSH * FWORDS] \
        .rearrange("o (r w) -> (o r) w", w=FWORDS)         # [160, 2048] i32
    ixp_ap = blob_d.ap()[0:1, SMO:SMO + 20480].bitcast(dt.int16) \
        .rearrange("o (r s) -> (o r) s", s=256)            # [160, 256] i16
    hmr_ap = blob_d.ap()[0:1, SMO + 20480:SMO + 36864] \
        .rearrange("o (p q) -> (o p) q", q=H * T)          # [128, 128] i32
    shw_ap = blob_d.ap()[0:1, SMO + 36864:SMO + 40960] \
        .rearrange("o (p q) -> (o p) q", q=T)              # [128, 32] i32
    msk_ap = blob_d.ap()[0:1, SMO + 40960:SMO + 43008] \
        .rearrange("o (p q) -> (o p) q", q=16)             # [128, 16] i32
    idn_ap = blob_d.ap()[0:1, SMO + 43008:SMO + 59392].bitcast(dt.float32) \
        .rearrange("o (p q) -> (o p) q", q=128)            # [128, 128] f32
    one_ap = blob_d.ap()[0:1, SMO + 59392:SMO + 59520].bitcast(dt.float32) \
        .rearrange("o (p q) -> (o p) q", q=1)              # [128, 1] f32

    with tile.TileContext(nc) as tc:
        with ExitStack() as ctx:
            dram = ctx.enter_context(tc.tile_pool(name="dram", bufs=1,
                                                  space="DRAM"))
            pool = ctx.enter_context(tc.tile_pool(name="main", bufs=1))
            work = ctx.enter_context(tc.tile_pool(name="work", bufs=2))
            ppool = ctx.enter_context(tc.tile_pool(name="ps", bufs=2,
                                                   space="PSUM"))

            # --- filters: shard -> AllGather -> SBUF-resident packed tables
            # The emulated runtime only delivers the first half of each
            # cross-pair contribution, so send the shard twice and read the
            # (complete) first half of each core's doubled contribution.
            fb2 = dram.tile([2 * FSH, FWORDS], dt.int32)
            ffx = nc.dram_tensor("ffx", [2 * C * N, FWORDS], dt.int32,
                                 addr_space="Shared")
            nc.gpsimd.dma_start(fb2[0:FSH, :], fw_ap)
            nc.gpsimd.dma_start(fb2[FSH:2 * FSH, :], fw_ap)
            nc.gpsimd.collective_compute(
                "AllGather", mybir.AluOpType.bypass,
                replica_groups=[list(range(NCORES))],
                ins=[fb2[:].opt()], outs=[ffx.ap().opt()])

            filt = []
            for c in range(C):
                ft = pool.tile([128, FWORDS], dt.int32, name=f"filt{c}")
                # global packed row r -> core k=r//FSH at ffx row 2*FSH*k + r%FSH
                r = c * N
                while r < (c + 1) * N:
                    k = r // FSH
                    end = min((c + 1) * N, (k + 1) * FSH)
                    nc.sync.dma_start(
                        ft[r - c * N:end - c * N, :],
                        ffx.ap()[2 * FSH * k + (r % FSH):
                                 2 * FSH * k + (r % FSH) + (end - r), :])
                    r = end
                filt.append(ft)

            # --- small constants (one packed buffer)
            ixp = []
            for c in range(C):
                it_ = pool.tile([128, 256], dt.int16, name=f"ixp{c}")
                for g in range(8):
                    nc.sync.dma_start(it_[16 * g:16 * (g + 1), :],
                                      ixp_ap[c * 16:(c + 1) * 16, :])
                ixp.append(it_)
            hmr = pool.tile([128, H * T], dt.int32, name="hmr")
            nc.sync.dma_start(hmr[:], hmr_ap)
            shw = pool.tile([128, T], dt.int32, name="shw")
            nc.sync.dma_start(shw[:], shw_ap)
            msk = pool.tile([128, 16], dt.int32, name="msk")
            nc.sync.dma_start(msk[:], msk_ap)
            idn = pool.tile([128, 128], dt.float32, name="idn")
            nc.sync.dma_start(idn[:], idn_ap)
            one = pool.tile([128, 1], dt.float32, name="one")
            nc.sync.dma_start(one[:], one_ap)

            for t in range(NCHUNK):
                # unpack this chunk's sample words into bit-granular table
                wtab = work.tile([128, SWORDS], dt.int32, name="wtab", bufs=1)
                nc.sync.dma_start(wtab[:], _sw_ap(t))
                btab = work.tile([128, ENTRY], dt.int32, name="btab", bufs=1)
                b3t = btab[:].rearrange("p (w j) -> p w j", j=T)
                wv = wtab[:].unsqueeze(2).broadcast_to([128, SWORDS, T])
                sv = shw[:].unsqueeze(1).broadcast_to([128, SWORDS, T])
                nc.vector.tensor_tensor(
                    b3t, wv, sv, mybir.AluOpType.logical_shift_right)
                nc.vector.tensor_scalar(
                    btab[:], btab[:], 1, None, mybir.AluOpType.bitwise_and)
                for c in range(C):
                    # 1) permutation gather: bit-granular, idx = tm directly
                    bits = work.tile([128, ENTRY], dt.int32, name="bits",
                                     bufs=1)
                    nc.gpsimd.ap_gather(
                        bits[:], btab[:], ixp[c][:],
                        channels=128, num_elems=ENTRY, d=1, num_idxs=ENTRY)
                    # 2) H3 hash
                    idxt = work.tile([128, 512], dt.int16, name="idxt", bufs=1)
                    shnt = work.tile([128, 512], dt.int32, name="shnt", bufs=1)
                    tk = work.tile([128, ENTRY], dt.int32, name="tk", bufs=1)
                    b3 = bits[:].rearrange("p (n j) -> p n j", j=T)
                    t3 = tk[:].rearrange("p (n j) -> p n j", j=T)
                    for k in range(H):
                        hk = work.tile([128, 128], dt.int32, name="hk", bufs=1)
                        hmk = hmr[:, k * T:(k + 1) * T].unsqueeze(1) \
                            .broadcast_to([128, N, T])
                        nc.vector.tensor_tensor(t3, b3, hmk,
                                                mybir.AluOpType.mult)
                        nc.vector.tensor_reduce(
                            hk[:], t3, mybir.AxisListType.X,
                            mybir.AluOpType.bitwise_xor)
                        # 3) split to word idx / bit shift, f32, PE-transpose
                        wf = work.tile([128, 128], dt.int32, name="wf", bufs=1)
                        nc.vector.tensor_scalar(
                            wf[:], hk[:], 5, None,
                            mybir.AluOpType.logical_shift_right)
                        wff = work.tile([128, 128], dt.float32, name="wff",
                                        bufs=1)
                        nc.scalar.copy(wff[:], wf[:])
                        sf = work.tile([128, 128], dt.int32, name="sf", bufs=1)
                        nc.vector.tensor_scalar(
                            sf[:], hk[:], 31, None, mybir.AluOpType.bitwise_and)
                        sff = work.tile([128, 128], dt.float32, name="sff",
                                        bufs=1)
                        nc.scalar.copy(sff[:], sf[:])
                        tw = ppool.tile([128, 128], dt.float32, name="tw")
                        nc.tensor.matmul(tw[:], wff[:], idn[:],
                                         is_transpose=True)
                        ts = ppool.tile([128, 128], dt.float32, name="ts")
                        nc.tensor.matmul(ts[:], sff[:], idn[:],
                                         is_transpose=True)
                        iv = idxt[:].rearrange("p (b k) -> p b k", k=H)
                        nc.vector.tensor_copy(iv[:, :, k], tw[:])
                        sv = shnt[:].rearrange("p (b k) -> p b k", k=H)
                        nc.scalar.copy(sv[:, :, k], ts[:])
                    # 4) bloom probes, two halves of 64 samples
                    for hf in range(2):
                        gout = work.tile([128, 4096], dt.int32, name="gout")
                        nc.gpsimd.ap_gather(
                            gout[:], filt[c][:],
                            idxt[:, hf * 256:(hf + 1) * 256],
                            channels=128, num_elems=FWORDS, d=1, num_idxs=4096)
                        gv = gout[:].rearrange("p (s j) -> p s j", j=16)
                        mb = msk[:].unsqueeze(1).broadcast_to([128, 256, 16])
                        wsel = work.tile([128, 256], dt.int32, name="wsel",
                                         bufs=1)
                        nc.vector.tensor_tensor(
                            gv, gv, mb, mybir.AluOpType.bitwise_and)
                        nc.vector.tensor_reduce(
                            wsel[:], gv, mybir.AxisListType.X,
                            mybir.AluOpType.bitwise_or)
                        nc.vector.tensor_tensor(
                            wsel[:], wsel[:],
                            shnt[:, hf * 256:(hf + 1) * 256],
                            mybir.AluOpType.logical_shift_right)
                        nc.vector.tensor_scalar(
                            wsel[:], wsel[:], 1, None,
                            mybir.AluOpType.bitwise_and)
                        # AND over the 4 hash fns
                        mem = work.tile([128, 64], dt.int32, name="mem",
                                        bufs=1)
                        nc.vector.tensor_reduce(
                            mem[:], wsel[:].rearrange("p (b k) -> p b k", k=H),
                            mybir.AxisListType.X, mybir.AluOpType.bitwise_and)
                        memf = work.tile([128, 64], dt.float32, name="memf",
                                         bufs=1)
                        nc.vector.tensor_copy(memf[:], mem[:])
                        # 5) count over neurons (partition reduce via matmul)
                        pr = ppool.tile([1, 64], dt.float32, name="pr")
                        nc.tensor.matmul(pr[:], one[:], memf[:],
                                         start=True, stop=True)
                        sr = work.tile([1, 64], dt.float32, name="sr")
                        nc.scalar.copy(sr[:], pr[:])
                        off = c * BC + t * 128 + hf * 64
                        nc.sync.dma_start(resp_d.ap()[0:1, off:off + 64],
                                          sr[:])
    nc.compile()
    return nc


def _get_runner():
    if "runner" in _CACHE:
        return _CACHE["runner"]
    import jax
    import numpy as _np
    from jax.sharding import Mesh, PartitionSpec
    from jax.experimental.shard_map import shard_map
    from concourse import bass2jax

    nc = _build_nc()
    bass2jax.install_neuronx_cc_hook()
    devices = jax.devices()[:NCORES]
    mesh = Mesh(_np.asarray(devices), ("core",))
    out_avals = [jax.core.ShapedArray((1, C * BC), _np.float32)]
    partition_name = (nc.partition_id_tensor.name
                      if nc.partition_id_tensor else None)
    in_names = ["blob", "resp"]
    if partition_name:
        in_names.append(partition_name)

    def _body(blob, zout):
        operands = [blob, zout]
        if partition_name:
            operands.append(bass2jax.partition_id_tensor())
        outs = bass2jax._bass_exec_p.bind(
            *operands,
            out_avals=tuple(out_avals),
            in_names=tuple(in_names),
            out_names=("resp",),
            lowering_input_output_aliases=(),
            sim_require_finite=True,
            sim_require_nnan=True,
            nc=nc)
        return tuple(outs)

    P = PartitionSpec
    in_specs = (P("core"), P("core"))
    runner = jax.jit(
        shard_map(_body, mesh=mesh, in_specs=in_specs,
                  out_specs=(P("core"),), check_rep=False),
        donate_argnums=(1,), keep_unused=True)
    _CACHE["runner"] = (runner, mesh)
    return _CACHE["runner"]


_PK_W = None


def _pack_bits_f32(arr_f32_flat32):
    """[M, 32] f32 of 0/1 -> [M] uint32 via BLAS gemm (exact in two 16-bit
    halves)."""
    global _PK_W
    if _PK_W is None:
        W = np.zeros((32, 2), np.float32)
        W[:16, 0] = (1 << np.arange(16)).astype(np.float32)
        W[16:, 1] = (1 << np.arange(16)).astype(np.float32)
        _PK_W = W
    lohi = arr_f32_flat32 @ _PK_W
    return lohi[:, 0].astype(np.uint32) | (lohi[:, 1].astype(np.uint32) << 16)


def _get_packers():
    """Numba single-pass bit packers; fall back to the gemm path."""
    if "packers" in _CACHE:
        return _CACHE["packers"]
    try:
        from numba import njit

        @njit(cache=False, nogil=True)
        def _pk_i32(src, out):
            # src int32 [M, 32] of 0/1 -> out uint32 [M]
            for i in range(src.shape[0]):
                w = np.uint32(0)
                for t in range(32):
                    w |= np.uint32(src[i, t]) << np.uint32(t)
                out[i] = w

        @njit(cache=False, nogil=True)
        def _pk_f32v(src, out):
            # src uint32-view of f32 0.0/1.0 [M, 32]; 1.0f has bit 29 set
            for i in range(src.shape[0]):
                w = np.uint32(0)
                for t in range(32):
                    w |= ((src[i, t] >> np.uint32(29)) & np.uint32(1)) \
                        << np.uint32(t)
                out[i] = w

        # trigger compilation now (first, untimed call)
        _pk_i32(np.zeros((2, 32), np.int32), np.empty(2, np.uint32))
        _pk_f32v(np.zeros((2, 32), np.uint32), np.empty(2, np.uint32))

        def pack_samples(samples):
            out = np.empty(B * SWORDS, np.uint32)
            _pk_i32(samples.reshape(-1, 32), out)
            return out.reshape(B, SWORDS).view(np.int32)

        def pack_filters(filters):
            out = np.empty(C * N * FWORDS, np.uint32)
            _pk_f32v(np.ascontiguousarray(filters).reshape(-1, 32)
                     .view(np.uint32), out)
            return out.reshape(C * N, FWORDS).view(np.int32)
    except Exception:
        def pack_samples(samples):
            sf = samples.astype(np.float32)
            return _pack_bits_f32(sf.reshape(-1, 32)).reshape(
                B, SWORDS).view(np.int32)

        def pack_filters(filters):
            fwv = _pack_bits_f32(np.ascontiguousarray(
                filters.reshape(-1, 32).astype(np.float32, copy=False)))
            return fwv.reshape(C * N, FWORDS).view(np.int32)

    _CACHE["packers"] = (pack_samples, pack_filters)
    return _CACHE["packers"]


def kernel(samples, tuple_mapping, hash_matrix, filters):
    samples = np.asarray(samples)
    tm = np.asarray(tuple_mapping).astype(np.int64)
    hm = np.asarray(hash_matrix).astype(np.int64)
    filters = np.asarray(filters)

    runner, mesh = _get_runner()
    pack_samples, pack_filters = _get_packers()

    # --- pack filters (f32 0/1 -> u32 words)
    fw = pack_filters(filters)

    # --- small constants, packed into one int32 buffer
    ixp = np.empty((C * 16, 256), np.int16)
    for c in range(C):
        # wrapped idx layout: stream pos i -> (partition i%16, slot i//16)
        ixp[c * 16:(c + 1) * 16, :] = tm[c].astype(np.int16).reshape(256, 16).T
    hmr = np.tile(hm.reshape(1, H * T).astype(np.int32), (128, 1))
    shw = np.tile(np.arange(T, dtype=np.int32)[None, :], (128, 1))
    pp = np.arange(128)
    msk = -(np.arange(16)[None, :] == (pp[:, None] & 15)).astype(np.int32)
    idn = np.eye(128, dtype=np.float32)
    one = np.ones((128, 1), np.float32)
    sm = np.concatenate([
        ixp.reshape(-1).view(np.int32),
        hmr.reshape(-1), shw.reshape(-1), msk.reshape(-1),
        idn.reshape(-1).view(np.int32), one.reshape(-1).view(np.int32),
    ]).reshape(1, SM_LEN)

    sw = pack_samples(samples)

    blob = np.concatenate([
        sw.reshape(NCORES, BC * SWORDS),
        fw.reshape(NCORES, FSH * FWORDS),
        np.broadcast_to(sm, (NCORES, SM_LEN)),
    ], axis=1)

    zout = np.zeros((NCORES, C * BC), np.float32)
    outs = runner(blob, zout)
    resp = np.asarray(outs[0])                             # [8, C*BC]

    out = np.empty((B, C), np.float32)
    r = resp.reshape(NCORES, C, BC)
    for core in range(NCORES):
        out[core * BC:(core + 1) * BC, :] = r[core].T
    return out
